# revision 10
# baseline (speedup 1.0000x reference)
"""Cross-attention Bass/Tile kernel for Trainium2, data-parallel over batch on 8 cores.

Problem (hardcoded): x_1 [2048,16,100], x_2 [2048,16,100], Wq/Wk/Wv [100,128], fp32.
  Q = x1 @ Wq; K = x2 @ Wk; V = x2 @ Wv  (per batch)
  out = softmax(Q K^T / sqrt(128)) @ V   -> [2048,16,128]

Sharding: batch dim split 8 ways (2 batches per core). Full inputs in, full output out.

v2 design notes (ACT-engine paced):
  The per-core floor is the scalar/ACT engine: 2*S*S = 8.4M exps at 1 elem/lane/cycle
  @1.2GHz = 54.6us + per-instr bubbles -> ~68-71us for 64 [128,1024] EXPs. PE issue
  work (S^T + PV + prep) is ~65us. So ACT must do NOTHING but the exps, back-to-back:
   - x loaded fp32 via 8 batched DMAs (rearrange, 4 t-tiles each); PE transposes fp32
     directly (1 col/cycle, trace-verified); Pool evicts+casts psum->bf16. No input
     casts on ACT.
   - tail normalize-muls on Pool (tensor_scalar_mul), psum evictions on Pool/DVE.
   - rowsum tree on DVE all-16-bit (fp16 mid levels keep the 2x DVE mode).
   - pending tails emitted at TOP of each chunk: baseline emitted them after the
     next chunk's pair loop, making the last chunk's self-PV matmuls wait on a PSUM
     WAR hazard (6.1us end stall in the trace).
   - EXP activation table preloaded via a dummy [128,1] exp during ramp.
   - one output DMA per chunk ([128,4,128] rearrange), engine-issue cost ~600ns each.

Per-core dataflow (2 batches b0,b1; 4 chunks of 512 s-cols each; pairs g = 2 t-tiles):
  S^T pair [128,1024] fp32 psum (2 matmuls) -> ACT exp*scale -> et bf16 SBUF
  PV of prev chunk (2 accum matmuls) interleaved after each exp; last chunk self-PV.
  rowsum: DVE tree et[8192] -> fp16 -> rowacc bf16 [128,512]
  tail: Pool evict O^T, PE ones-matmuls (denominators), DVE recip, PE transposes,
        Pool scale, sync DMA out.
  Ramp: b0 prep (fp32 transposes + QK projections) interleaved just-in-time into
  chunk 0's pair loop; b1 prep during chunks 1-2.
"""

import sys

sys.path.insert(0, "/opt/trn_rl_repo")

import numpy as np

import concourse.bass as bass
import concourse.tile as tile
from concourse import bacc, mybir
from concourse.bass_utils import run_bass_kernel_spmd
from concourse.masks import make_identity

S = 2048
B = 16
DH = 100
DK = 128
NCORES = 8
BPC = B // NCORES
F32 = mybir.dt.float32
BF16 = mybir.dt.bfloat16
FP16 = mybir.dt.float16
SCALE = 1.0 / float(np.sqrt(np.float32(DK)))

ST = S // 128     # 16 t-tiles of 128
NSC = S // 512    # 4 chunks of 512 per batch
XCOLS = ST * 2 * DH + 32  # [128, 3232]: 16 tiles x 200 cols + pad for b1/k15 window


def _attention_kernel(tc, out, x1, x2, wq, wk, wv):
    nc = tc.nc

    with (
        tc.tile_pool(name="const", bufs=1) as constp,
        tc.tile_pool(name="xn", bufs=2) as xnp,
        tc.tile_pool(name="xT", bufs=4) as xtp,
        tc.tile_pool(name="qk", bufs=4) as qkp,
        tc.tile_pool(name="vp", bufs=2) as vp,
        tc.tile_pool(name="et", bufs=2) as etp,
        tc.tile_pool(name="acc", bufs=2) as accp,
        tc.tile_pool(name="rowb", bufs=2) as rowbp,
        tc.tile_pool(name="rr", bufs=2) as rrp,
        tc.tile_pool(name="osb", bufs=2) as osbp,
        tc.tile_pool(name="osc", bufs=2) as oscp,
        tc.tile_pool(name="ps_st", bufs=2, space="PSUM") as psb,
        tc.tile_pool(name="ps_ot", bufs=2, space="PSUM") as psot,
        tc.tile_pool(name="ps_sc", bufs=2, space="PSUM") as pssc,
    ):
        # ---- input DMAs first (sync: x2 groups + weights; gpsimd: x1 g0;
        # vector: x1 g1-3). Each group DMA lands 4 t-tiles [128, 4x200] fp32.
        xn_tiles = {}
        for src_i in (0, 1):
            xn_tiles[src_i] = xnp.tile([128, XCOLS], F32, tag="xn", name=f"xn{src_i}")

        def x_group_dma(eng, src_i, src_ap, g):
            src = src_ap[g * 512:(g + 1) * 512, :, :].rearrange(
                "(k p) b d -> p k (b d)", k=4
            )
            eng.dma_start(xn_tiles[src_i][:, g * 800:(g + 1) * 800], src)

        w_f32s = {}
        x_group_dma(nc.sync, 1, x2, 0)
        w_f32s["wk"] = constp.tile([DH, DK], F32, name="wk_f32")
        nc.sync.dma_start(w_f32s["wk"], wk)
        x_group_dma(nc.gpsimd, 0, x1, 0)
        w_f32s["wq"] = constp.tile([DH, DK], F32, name="wq_f32")
        nc.sync.dma_start(w_f32s["wq"], wq)
        x_group_dma(nc.sync, 1, x2, 1)
        w_f32s["wv"] = constp.tile([DH, DK], F32, name="wv_f32")
        nc.sync.dma_start(w_f32s["wv"], wv)
        x_group_dma(nc.scalar, 0, x1, 1)
        x_group_dma(nc.sync, 1, x2, 2)
        x_group_dma(nc.sync, 1, x2, 3)
        x_group_dma(nc.scalar, 0, x1, 2)
        x_group_dma(nc.scalar, 0, x1, 3)

        # ---- constants
        ident = constp.tile([128, 128], F32)
        make_identity(nc, ident)
        ones_bf = constp.tile([128, 1], BF16)
        nc.vector.memset(ones_bf, 1.0)
        w_sbs = {}
        for wname in ("wk", "wq", "wv"):
            w_sb = constp.tile([DH, DK], BF16, name=f"{wname}_sb")
            nc.vector.tensor_copy(w_sb, w_f32s[wname])
            w_sbs[wname] = w_sb
        wq_sb, wk_sb, wv_sb = w_sbs["wq"], w_sbs["wk"], w_sbs["wv"]
        # preload the EXP activation table during ramp (dummy exp)
        dum = constp.tile([128, 1], F32, name="dum")
        nc.vector.memset(dum, 0.0)
        dum_o = constp.tile([128, 1], BF16, name="dum_o")
        nc.scalar.activation(dum_o, dum, mybir.ActivationFunctionType.Exp)
        # zero the pad window read by the b=1, k=15 transpose slice
        for src_i in (0, 1):
            nc.gpsimd.memset(xn_tiles[src_i][:, ST * 2 * DH:], 0.0)

        # persistent transposed/projected tensors
        xTs, qTs, kTs, vas = {}, {}, {}, {}
        for src_i in (0, 1):
            for b in range(BPC):
                xTs[(src_i, b)] = xtp.tile(
                    [128, S], BF16, tag="xT", name=f"xT_{src_i}_{b}"
                )
        for b in range(BPC):
            qTs[b] = qkp.tile([DK, S], BF16, tag="qk", name=f"qT_{b}")
            kTs[b] = qkp.tile([DK, S], BF16, tag="qk", name=f"kT_{b}")
            vas[b] = vp.tile([128, S], BF16, tag="v", name=f"vall_{b}")

        def emit_xt_g(src_i, b, g):
            """Transpose 4 fp32 t-tiles on PE, evict+cast to bf16 xT on Pool."""
            psq = pssc.tile([128, 512], F32, tag="sc", name=f"xq_{src_i}_{b}_{g}")
            xn = xn_tiles[src_i]
            for j in range(4):
                tt = g * 4 + j
                c0 = tt * 2 * DH + b * DH
                nc.tensor.transpose(
                    psq[:, j * 128:(j + 1) * 128], xn[:, c0:c0 + 128], ident
                )
            nc.vector.tensor_copy(xTs[(src_i, b)][:, g * 512:(g + 1) * 512], psq)

        def emit_proj_c(dstT, w_sb, xT, b, c, eng=None):
            csl = slice(c * 512, (c + 1) * 512)
            pj = pssc.tile([128, 512], F32, tag="sc", name=f"pj_{b}_{c}")
            nc.tensor.matmul(pj, w_sb, xT[:DH, csl], start=True, stop=True)
            if eng is nc.scalar:
                nc.scalar.copy(dstT[:, csl], pj)
            else:
                nc.vector.tensor_copy(dstT[:, csl], pj)

        def emit_prep_qk(b):
            for src_i in (0, 1):
                for g in range(4):
                    emit_xt_g(src_i, b, g)
            for dstT, w_sb, xT in (
                (qTs[b], wq_sb, xTs[(0, b)]), (kTs[b], wk_sb, xTs[(1, b)])
            ):
                for c in range(NSC):
                    emit_proj_c(dstT, w_sb, xT, b, c)

        def emit_prep_v(b):
            x2T = xTs[(1, b)]
            for g in range(4):
                psv = pssc.tile([128, 512], F32, tag="sc", name=f"vg_{b}_{g}")
                for j in range(4):
                    tt = g * 4 + j
                    nc.tensor.matmul(
                        psv[:, j * 128:(j + 1) * 128],
                        x2T[:DH, tt * 128:(tt + 1) * 128],
                        wv_sb,
                        start=True, stop=True,
                    )
                nc.vector.tensor_copy(vas[b][:, g * 512:(g + 1) * 512], psv)

        def emit_tail(st_):
            b, sc, rowacc, otp = st_
            ot_sb = osbp.tile([128, 512], F32, tag="osb", name=f"otsb_{b}_{sc}")
            nc.vector.tensor_copy(ot_sb, otp)
            rs_all = pssc.tile([128, 4], F32, tag="sc", name=f"rs_{b}_{sc}")
            for si in range(4):
                nc.tensor.matmul(
                    rs_all[:, si:si + 1],
                    rowacc[:, si * 128:(si + 1) * 128], ones_bf,
                    start=True, stop=True,
                )
            rr_all = rrp.tile([128, 4], F32, tag="rr", name=f"rr_{b}_{sc}")
            nc.vector.reciprocal(rr_all, rs_all)
            otr_all = pssc.tile([128, 512], F32, tag="sc", name=f"otr_{b}_{sc}")
            osc_all = oscp.tile([128, 512], F32, tag="osc", name=f"osc_{b}_{sc}")
            for si in range(4):
                scol = slice(si * 128, (si + 1) * 128)
                nc.tensor.transpose(otr_all[:, scol], ot_sb[:, scol], ident)
                nc.vector.tensor_scalar_mul(
                    osc_all[:, scol], otr_all[:, scol], rr_all[:, si:si + 1]
                )
            dst = out[sc * 512:(sc + 1) * 512, b, :].rearrange(
                "(k p) d -> p k d", k=4
            )
            nc.sync.dma_start(dst, osc_all)

        # ---- main loop: 8 chunks; PV of chunk i-1 interleaves into chunk i.
        items = [(b, sc) for b in range(BPC) for sc in range(NSC)]
        prev = None          # (b, sc, et, rowacc, vall)
        pending_tail = None
        otp_self = None
        acch_last = None
        for idx, (b, sc) in enumerate(items):
            if idx == 1:
                emit_prep_qk(1)
            if idx == 2:
                emit_prep_v(1)
            # tails first: frees the psot buf before this chunk claims it
            if pending_tail is not None:
                emit_tail(pending_tail)
                pending_tail = None
            qT, kT = qTs[b], kTs[b]
            ssl = slice(sc * 512, (sc + 1) * 512)
            et = etp.tile([128, ST * 512], BF16, tag="et", name=f"et_{b}_{sc}")
            if prev is not None:
                potp = psot.tile([128, 512], F32, tag="ot",
                                 name=f"ot_{prev[0]}_{prev[1]}")
            is_last = idx == len(items) - 1
            if is_last:
                otp_self = psot.tile([128, 512], F32, tag="ot", name=f"ot_{b}_{sc}")
                acch_last = accp.tile([128, 4096], FP16, tag="acc",
                                      name=f"acch_{b}_{sc}")
            for g in range(ST // 2):
                if idx == 0:
                    # just-in-time b0 prep interleave
                    if g == 0:
                        emit_xt_g(1, 0, 0)
                        emit_proj_c(kTs[0], wk_sb, xTs[(1, 0)], 0, 0)
                        emit_xt_g(0, 0, 0)
                        emit_proj_c(qTs[0], wq_sb, xTs[(0, 0)], 0, 0,
                                    eng=nc.scalar)
                    elif g in (1, 3, 5):
                        gp = (g + 1) // 2
                        emit_xt_g(1, 0, gp)
                        emit_proj_c(kTs[0], wk_sb, xTs[(1, 0)], 0, gp)
                ps = psb.tile([128, 1024], F32, tag="st", name=f"st_{b}_{sc}_{g}")
                for h in range(2):
                    tt = g * 2 + h
                    nc.tensor.matmul(
                        ps[:, h * 512:(h + 1) * 512],
                        kT[:, tt * 128:(tt + 1) * 128],
                        qT[:, ssl],
                        start=True, stop=True,
                    )
                nc.scalar.activation(
                    et[:, g * 1024:(g + 1) * 1024], ps,
                    mybir.ActivationFunctionType.Exp, scale=SCALE,
                )
                if prev is not None:
                    pb, psc, pet, prow, pvall = prev
                    for h in range(2):
                        ptt = g * 2 + h
                        nc.tensor.matmul(
                            potp,
                            pvall[:, ptt * 128:(ptt + 1) * 128],
                            pet[:, ptt * 512:(ptt + 1) * 512],
                            start=(ptt == 0),
                            stop=(ptt == ST - 1),
                        )
                if is_last:
                    for h in range(2):
                        tt = g * 2 + h
                        nc.tensor.matmul(
                            otp_self,
                            vas[b][:, tt * 128:(tt + 1) * 128],
                            et[:, tt * 512:(tt + 1) * 512],
                            start=(tt == 0),
                            stop=(tt == ST - 1),
                        )
                    # incremental rowsum to shorten the drain
                    if g == 3:
                        nc.vector.tensor_add(
                            acch_last[:, :2048], et[:, :2048], et[:, 2048:4096]
                        )
                    elif g == 5:
                        nc.vector.tensor_add(
                            acch_last[:, :2048], acch_last[:, :2048],
                            et[:, 4096:6144],
                        )
            if idx == 0:
                for gp in range(1, 4):
                    emit_xt_g(0, 0, gp)
                    emit_proj_c(qTs[0], wq_sb, xTs[(0, 0)], 0, gp)
                emit_prep_v(0)
            # rowsum: DVE tree, all 16-bit to keep the 2x perf mode
            rowacc = rowbp.tile([128, 512], BF16, tag="rowb", name=f"row_{b}_{sc}")
            if is_last:
                nc.vector.tensor_add(
                    acch_last[:, :2048], acch_last[:, :2048], et[:, 6144:8192]
                )
                nc.vector.tensor_add(
                    acch_last[:, :1024], acch_last[:, :1024],
                    acch_last[:, 1024:2048],
                )
                nc.vector.tensor_add(
                    rowacc, acch_last[:, :512], acch_last[:, 512:1024]
                )
            else:
                # rowsum on Pool (SBUF-only engine): keeps DVE free for psum
                # evictions; Pool is otherwise idle in steady state
                acch = accp.tile([128, 4096], FP16, tag="acc", name=f"acch_{b}_{sc}")
                nc.gpsimd.tensor_add(acch, et[:, :4096], et[:, 4096:])
                nc.gpsimd.tensor_add(acch[:, :2048], acch[:, :2048], acch[:, 2048:])
                nc.gpsimd.tensor_add(acch[:, :1024], acch[:, :1024],
                                     acch[:, 1024:2048])
                nc.gpsimd.tensor_add(rowacc, acch[:, :512], acch[:, 512:1024])
            if prev is not None:
                pending_tail = (prev[0], prev[1], prev[3], potp)
            prev = (b, sc, et, rowacc, vas[b])
        # drain: tails of the final two chunks
        pb, psc, pet, prow, pvall = prev
        if pending_tail is not None:
            emit_tail(pending_tail)
        emit_tail((pb, psc, prow, otp_self))


_NC_CACHE = None


def _build():
    global _NC_CACHE
    if _NC_CACHE is not None:
        return _NC_CACHE
    nc = bacc.Bacc("TRN2", target_bir_lowering=False, debug=False, num_devices=NCORES)
    x1 = nc.dram_tensor("x_1", (S, BPC, DH), F32, kind="ExternalInput").ap()
    x2 = nc.dram_tensor("x_2", (S, BPC, DH), F32, kind="ExternalInput").ap()
    wq = nc.dram_tensor("Wq", (DH, DK), F32, kind="ExternalInput").ap()
    wk = nc.dram_tensor("Wk", (DH, DK), F32, kind="ExternalInput").ap()
    wv = nc.dram_tensor("Wv", (DH, DK), F32, kind="ExternalInput").ap()
    out = nc.dram_tensor("out", (S, BPC, DK), F32, kind="ExternalOutput").ap()
    with tile.TileContext(nc) as tc:
        _attention_kernel(tc, out, x1, x2, wq, wk, wv)
    nc.compile()
    _NC_CACHE = nc
    return nc


def _in_maps(x_1, x_2, Wq, Wk, Wv):
    maps = []
    for c in range(NCORES):
        bsl = slice(c * BPC, (c + 1) * BPC)
        maps.append({
            "x_1": np.ascontiguousarray(x_1[:, bsl, :], dtype=np.float32),
            "x_2": np.ascontiguousarray(x_2[:, bsl, :], dtype=np.float32),
            "Wq": np.asarray(Wq, dtype=np.float32),
            "Wk": np.asarray(Wk, dtype=np.float32),
            "Wv": np.asarray(Wv, dtype=np.float32),
        })
    return maps


def run(x_1, x_2, Wq, Wk, Wv, **spmd_kwargs):
    nc = _build()
    in_maps = _in_maps(x_1, x_2, Wq, Wk, Wv)
    last_err = None
    for _attempt in range(3):
        try:
            res = run_bass_kernel_spmd(
                nc, in_maps, core_ids=list(range(NCORES)), **spmd_kwargs
            )
            break
        except Exception as e:  # transient NRT_EXEC_UNIT_UNRECOVERABLE faults
            last_err = e
    else:
        raise last_err
    out = np.concatenate([res.results[c]["out"] for c in range(NCORES)], axis=1)
    return out, res


def kernel(x_1, x_2, Wq, Wk, Wv):
    out, _ = run(x_1, x_2, Wq, Wk, Wv)
    return out.astype(np.float32)


# revision 11
# speedup vs baseline: 1.1197x; 1.1197x over previous
"""Cross-attention Bass/Tile kernel for Trainium2, data-parallel over batch on 8 cores.

Problem (hardcoded): x_1 [2048,16,100], x_2 [2048,16,100], Wq/Wk/Wv [100,128], fp32.
  Q = x1 @ Wq; K = x2 @ Wk; V = x2 @ Wv  (per batch)
  out = softmax(Q K^T / sqrt(128)) @ V   -> [2048,16,128]

Sharding: batch dim split 8 ways (2 batches per core). Full inputs in, full output out.

v2 design notes (ACT-engine paced):
  The per-core floor is the scalar/ACT engine: 2*S*S = 8.4M exps at 1 elem/lane/cycle
  @1.2GHz = 54.6us + per-instr bubbles -> ~68-71us for 64 [128,1024] EXPs. PE issue
  work (S^T + PV + prep) is ~65us. So ACT must do NOTHING but the exps, back-to-back:
   - x loaded fp32 via 8 batched DMAs (rearrange, 4 t-tiles each); PE transposes fp32
     directly (1 col/cycle, trace-verified); Pool evicts+casts psum->bf16. No input
     casts on ACT.
   - tail normalize-muls on Pool (tensor_scalar_mul), psum evictions on Pool/DVE.
   - rowsum tree on DVE all-16-bit (fp16 mid levels keep the 2x DVE mode).
   - pending tails emitted at TOP of each chunk: baseline emitted them after the
     next chunk's pair loop, making the last chunk's self-PV matmuls wait on a PSUM
     WAR hazard (6.1us end stall in the trace).
   - EXP activation table preloaded via a dummy [128,1] exp during ramp.
   - one output DMA per chunk ([128,4,128] rearrange), engine-issue cost ~600ns each.

Per-core dataflow (2 batches b0,b1; 4 chunks of 512 s-cols each; pairs g = 2 t-tiles):
  S^T pair [128,1024] fp32 psum (2 matmuls) -> ACT exp*scale -> et bf16 SBUF
  PV of prev chunk (2 accum matmuls) interleaved after each exp; last chunk self-PV.
  rowsum: DVE tree et[8192] -> fp16 -> rowacc bf16 [128,512]
  tail: Pool evict O^T, PE ones-matmuls (denominators), DVE recip, PE transposes,
        Pool scale, sync DMA out.
  Ramp: b0 prep (fp32 transposes + QK projections) interleaved just-in-time into
  chunk 0's pair loop; b1 prep during chunks 1-2.
"""

import sys

sys.path.insert(0, "/opt/trn_rl_repo")

import numpy as np

import concourse.bass as bass
import concourse.tile as tile
from concourse import bacc, mybir
from concourse.bass_utils import run_bass_kernel_spmd
from concourse.masks import make_identity

S = 2048
B = 16
DH = 100
DK = 128
NCORES = 8
BPC = B // NCORES
F32 = mybir.dt.float32
BF16 = mybir.dt.bfloat16
FP16 = mybir.dt.float16
SCALE = 1.0 / float(np.sqrt(np.float32(DK)))

ST = S // 128     # 16 t-tiles of 128
NSC = S // 512    # 4 chunks of 512 per batch
XCOLS = ST * 2 * DH + 32  # [128, 3232]: 16 tiles x 200 cols + pad for b1/k15 window


def _attention_kernel(tc, out, x1, x2, wq, wk, wv):
    nc = tc.nc

    with (
        tc.tile_pool(name="const", bufs=1) as constp,
        tc.tile_pool(name="xn", bufs=2) as xnp,
        tc.tile_pool(name="xT", bufs=4) as xtp,
        tc.tile_pool(name="qk", bufs=4) as qkp,
        tc.tile_pool(name="vp", bufs=2) as vp,
        tc.tile_pool(name="et", bufs=2) as etp,
        tc.tile_pool(name="acc", bufs=2) as accp,
        tc.tile_pool(name="rowb", bufs=2) as rowbp,
        tc.tile_pool(name="rr", bufs=2) as rrp,
        tc.tile_pool(name="osb", bufs=2) as osbp,
        tc.tile_pool(name="osc", bufs=2) as oscp,
        tc.tile_pool(name="ps_st", bufs=2, space="PSUM") as psb,
        tc.tile_pool(name="ps_ot", bufs=2, space="PSUM") as psot,
        tc.tile_pool(name="ps_sc", bufs=2, space="PSUM") as pssc,
    ):
        # ---- input DMAs first (sync: x2 groups + weights; gpsimd: x1 g0;
        # vector: x1 g1-3). Each group DMA lands 4 t-tiles [128, 4x200] fp32.
        xn_tiles = {}
        for src_i in (0, 1):
            xn_tiles[src_i] = xnp.tile([128, XCOLS], F32, tag="xn", name=f"xn{src_i}")

        def x_group_dma(eng, src_i, src_ap, g):
            src = src_ap[g * 512:(g + 1) * 512, :, :].rearrange(
                "(k p) b d -> p k (b d)", k=4
            )
            eng.dma_start(xn_tiles[src_i][:, g * 800:(g + 1) * 800], src)

        w_f32s = {}
        x_group_dma(nc.sync, 1, x2, 0)
        w_f32s["wk"] = constp.tile([DH, DK], F32, name="wk_f32")
        nc.sync.dma_start(w_f32s["wk"], wk)
        x_group_dma(nc.gpsimd, 0, x1, 0)
        w_f32s["wq"] = constp.tile([DH, DK], F32, name="wq_f32")
        nc.sync.dma_start(w_f32s["wq"], wq)
        x_group_dma(nc.sync, 1, x2, 1)
        w_f32s["wv"] = constp.tile([DH, DK], F32, name="wv_f32")
        nc.sync.dma_start(w_f32s["wv"], wv)
        x_group_dma(nc.scalar, 0, x1, 1)
        x_group_dma(nc.sync, 1, x2, 2)
        x_group_dma(nc.sync, 1, x2, 3)
        x_group_dma(nc.scalar, 0, x1, 2)
        x_group_dma(nc.scalar, 0, x1, 3)

        # ---- constants
        ident = constp.tile([128, 128], F32)
        make_identity(nc, ident)
        ones_bf = constp.tile([128, 1], BF16)
        nc.vector.memset(ones_bf, 1.0)
        w_sbs = {}
        for wname in ("wk", "wq", "wv"):
            w_sb = constp.tile([DH, DK], BF16, name=f"{wname}_sb")
            nc.vector.tensor_copy(w_sb, w_f32s[wname])
            w_sbs[wname] = w_sb
        wq_sb, wk_sb, wv_sb = w_sbs["wq"], w_sbs["wk"], w_sbs["wv"]
        # preload the EXP activation table during ramp (dummy exp)
        dum = constp.tile([128, 1], F32, name="dum")
        nc.vector.memset(dum, 0.0)
        dum_o = constp.tile([128, 1], BF16, name="dum_o")
        nc.scalar.activation(dum_o, dum, mybir.ActivationFunctionType.Exp)
        # zero the pad window read by the b=1, k=15 transpose slice
        for src_i in (0, 1):
            nc.gpsimd.memset(xn_tiles[src_i][:, ST * 2 * DH:], 0.0)

        # persistent transposed/projected tensors
        xTs, qTs, kTs, vas = {}, {}, {}, {}
        for src_i in (0, 1):
            for b in range(BPC):
                xTs[(src_i, b)] = xtp.tile(
                    [128, S], BF16, tag="xT", name=f"xT_{src_i}_{b}"
                )
        for b in range(BPC):
            qTs[b] = qkp.tile([DK, S], BF16, tag="qk", name=f"qT_{b}")
            kTs[b] = qkp.tile([DK, S], BF16, tag="qk", name=f"kT_{b}")
            vas[b] = vp.tile([128, S], BF16, tag="v", name=f"vall_{b}")

        def emit_xt_g(src_i, b, g):
            """Transpose 4 fp32 t-tiles on PE, evict+cast to bf16 xT on Pool."""
            psq = pssc.tile([128, 512], F32, tag="sc", name=f"xq_{src_i}_{b}_{g}")
            xn = xn_tiles[src_i]
            for j in range(4):
                tt = g * 4 + j
                c0 = tt * 2 * DH + b * DH
                nc.tensor.transpose(
                    psq[:, j * 128:(j + 1) * 128], xn[:, c0:c0 + 128], ident
                )
            nc.vector.tensor_copy(xTs[(src_i, b)][:, g * 512:(g + 1) * 512], psq)

        def emit_proj_c(dstT, w_sb, xT, b, c, eng=None):
            csl = slice(c * 512, (c + 1) * 512)
            pj = pssc.tile([128, 512], F32, tag="sc", name=f"pj_{b}_{c}")
            nc.tensor.matmul(pj, w_sb, xT[:DH, csl], start=True, stop=True)
            if eng is nc.scalar:
                nc.scalar.copy(dstT[:, csl], pj)
            else:
                nc.vector.tensor_copy(dstT[:, csl], pj)

        def emit_prep_qk(b):
            for src_i in (0, 1):
                for g in range(4):
                    emit_xt_g(src_i, b, g)
            for dstT, w_sb, xT in (
                (qTs[b], wq_sb, xTs[(0, b)]), (kTs[b], wk_sb, xTs[(1, b)])
            ):
                for c in range(NSC):
                    emit_proj_c(dstT, w_sb, xT, b, c)

        def emit_prep_v(b):
            x2T = xTs[(1, b)]
            for g in range(4):
                psv = pssc.tile([128, 512], F32, tag="sc", name=f"vg_{b}_{g}")
                for j in range(4):
                    tt = g * 4 + j
                    nc.tensor.matmul(
                        psv[:, j * 128:(j + 1) * 128],
                        x2T[:DH, tt * 128:(tt + 1) * 128],
                        wv_sb,
                        start=True, stop=True,
                    )
                nc.vector.tensor_copy(vas[b][:, g * 512:(g + 1) * 512], psv)

        def emit_tail(st_):
            b, sc, rowacc, otp = st_
            ot_sb = osbp.tile([128, 512], F32, tag="osb", name=f"otsb_{b}_{sc}")
            nc.vector.tensor_copy(ot_sb, otp)
            rs_all = pssc.tile([128, 4], F32, tag="sc", name=f"rs_{b}_{sc}")
            for si in range(4):
                nc.tensor.matmul(
                    rs_all[:, si:si + 1],
                    rowacc[:, si * 128:(si + 1) * 128], ones_bf,
                    start=True, stop=True,
                )
            rr_all = rrp.tile([128, 4], F32, tag="rr", name=f"rr_{b}_{sc}")
            nc.vector.reciprocal(rr_all, rs_all)
            otr_all = pssc.tile([128, 512], F32, tag="sc", name=f"otr_{b}_{sc}")
            osc_all = oscp.tile([128, 512], F32, tag="osc", name=f"osc_{b}_{sc}")
            for si in range(4):
                scol = slice(si * 128, (si + 1) * 128)
                nc.tensor.transpose(otr_all[:, scol], ot_sb[:, scol], ident)
                nc.vector.tensor_scalar_mul(
                    osc_all[:, scol], otr_all[:, scol], rr_all[:, si:si + 1]
                )
            dst = out[sc * 512:(sc + 1) * 512, b, :].rearrange(
                "(k p) d -> p k d", k=4
            )
            nc.sync.dma_start(dst, osc_all)

        # ---- main loop: 8 chunks; PV of chunk i-1 interleaves into chunk i.
        items = [(b, sc) for b in range(BPC) for sc in range(NSC)]
        prev = None          # (b, sc, et, rowacc, vall)
        pending_tail = None
        otp_self = None
        acch_last = None
        for idx, (b, sc) in enumerate(items):
            if idx == 1:
                emit_prep_qk(1)
            if idx == 2:
                emit_prep_v(1)
            # tails first: frees the psot buf before this chunk claims it
            if pending_tail is not None:
                emit_tail(pending_tail)
                pending_tail = None
            qT, kT = qTs[b], kTs[b]
            ssl = slice(sc * 512, (sc + 1) * 512)
            et = etp.tile([128, ST * 512], BF16, tag="et", name=f"et_{b}_{sc}")
            if prev is not None:
                potp = psot.tile([128, 512], F32, tag="ot",
                                 name=f"ot_{prev[0]}_{prev[1]}")
            is_last = idx == len(items) - 1
            if is_last:
                otp_self = psot.tile([128, 512], F32, tag="ot", name=f"ot_{b}_{sc}")
                acch_last = accp.tile([128, 4096], FP16, tag="acc",
                                      name=f"acch_{b}_{sc}")
            for g in range(ST // 2):
                if idx == 0:
                    # just-in-time b0 prep interleave
                    if g == 0:
                        emit_xt_g(1, 0, 0)
                        emit_proj_c(kTs[0], wk_sb, xTs[(1, 0)], 0, 0)
                        emit_xt_g(0, 0, 0)
                        emit_proj_c(qTs[0], wq_sb, xTs[(0, 0)], 0, 0,
                                    eng=nc.scalar)
                    elif g in (1, 3, 5):
                        gp = (g + 1) // 2
                        emit_xt_g(1, 0, gp)
                        emit_proj_c(kTs[0], wk_sb, xTs[(1, 0)], 0, gp)
                ps = psb.tile([128, 1024], F32, tag="st", name=f"st_{b}_{sc}_{g}")
                for h in range(2):
                    tt = g * 2 + h
                    nc.tensor.matmul(
                        ps[:, h * 512:(h + 1) * 512],
                        kT[:, tt * 128:(tt + 1) * 128],
                        qT[:, ssl],
                        start=True, stop=True,
                    )
                nc.scalar.activation(
                    et[:, g * 1024:(g + 1) * 1024], ps,
                    mybir.ActivationFunctionType.Exp, scale=SCALE,
                )
                if prev is not None:
                    pb, psc, pet, prow, pvall = prev
                    for h in range(2):
                        ptt = g * 2 + h
                        nc.tensor.matmul(
                            potp,
                            pvall[:, ptt * 128:(ptt + 1) * 128],
                            pet[:, ptt * 512:(ptt + 1) * 512],
                            start=(ptt == 0),
                            stop=(ptt == ST - 1),
                        )
                if is_last:
                    for h in range(2):
                        tt = g * 2 + h
                        nc.tensor.matmul(
                            otp_self,
                            vas[b][:, tt * 128:(tt + 1) * 128],
                            et[:, tt * 512:(tt + 1) * 512],
                            start=(tt == 0),
                            stop=(tt == ST - 1),
                        )
                    # incremental rowsum to shorten the drain
                    if g == 3:
                        nc.vector.tensor_add(
                            acch_last[:, :2048], et[:, :2048], et[:, 2048:4096]
                        )
                    elif g == 5:
                        nc.vector.tensor_add(
                            acch_last[:, :2048], acch_last[:, :2048],
                            et[:, 4096:6144],
                        )
            if idx == 0:
                for gp in range(1, 4):
                    emit_xt_g(0, 0, gp)
                    emit_proj_c(qTs[0], wq_sb, xTs[(0, 0)], 0, gp)
                emit_prep_v(0)
            # rowsum: DVE tree, all 16-bit to keep the 2x perf mode
            rowacc = rowbp.tile([128, 512], BF16, tag="rowb", name=f"row_{b}_{sc}")
            if is_last:
                nc.vector.tensor_add(
                    acch_last[:, :2048], acch_last[:, :2048], et[:, 6144:8192]
                )
                nc.vector.tensor_add(
                    acch_last[:, :1024], acch_last[:, :1024],
                    acch_last[:, 1024:2048],
                )
                nc.vector.tensor_add(
                    rowacc, acch_last[:, :512], acch_last[:, 512:1024]
                )
            else:
                # L1 on DVE (2x mode, 16-bit); L2-L4 on Pool (SBUF-only engine,
                # ~0.5 elem/cycle/lane but otherwise idle in steady state)
                acch = accp.tile([128, 4096], FP16, tag="acc", name=f"acch_{b}_{sc}")
                nc.vector.tensor_add(acch, et[:, :4096], et[:, 4096:])
                nc.gpsimd.tensor_add(acch[:, :2048], acch[:, :2048], acch[:, 2048:])
                nc.gpsimd.tensor_add(acch[:, :1024], acch[:, :1024],
                                     acch[:, 1024:2048])
                nc.gpsimd.tensor_add(rowacc, acch[:, :512], acch[:, 512:1024])
            if prev is not None:
                pending_tail = (prev[0], prev[1], prev[3], potp)
            prev = (b, sc, et, rowacc, vas[b])
        # drain: tails of the final two chunks
        pb, psc, pet, prow, pvall = prev
        if pending_tail is not None:
            emit_tail(pending_tail)
        emit_tail((pb, psc, prow, otp_self))


_NC_CACHE = None


def _build():
    global _NC_CACHE
    if _NC_CACHE is not None:
        return _NC_CACHE
    nc = bacc.Bacc("TRN2", target_bir_lowering=False, debug=False, num_devices=NCORES)
    x1 = nc.dram_tensor("x_1", (S, BPC, DH), F32, kind="ExternalInput").ap()
    x2 = nc.dram_tensor("x_2", (S, BPC, DH), F32, kind="ExternalInput").ap()
    wq = nc.dram_tensor("Wq", (DH, DK), F32, kind="ExternalInput").ap()
    wk = nc.dram_tensor("Wk", (DH, DK), F32, kind="ExternalInput").ap()
    wv = nc.dram_tensor("Wv", (DH, DK), F32, kind="ExternalInput").ap()
    out = nc.dram_tensor("out", (S, BPC, DK), F32, kind="ExternalOutput").ap()
    with tile.TileContext(nc) as tc:
        _attention_kernel(tc, out, x1, x2, wq, wk, wv)
    nc.compile()
    _NC_CACHE = nc
    return nc


def _in_maps(x_1, x_2, Wq, Wk, Wv):
    maps = []
    for c in range(NCORES):
        bsl = slice(c * BPC, (c + 1) * BPC)
        maps.append({
            "x_1": np.ascontiguousarray(x_1[:, bsl, :], dtype=np.float32),
            "x_2": np.ascontiguousarray(x_2[:, bsl, :], dtype=np.float32),
            "Wq": np.asarray(Wq, dtype=np.float32),
            "Wk": np.asarray(Wk, dtype=np.float32),
            "Wv": np.asarray(Wv, dtype=np.float32),
        })
    return maps


def run(x_1, x_2, Wq, Wk, Wv, **spmd_kwargs):
    nc = _build()
    in_maps = _in_maps(x_1, x_2, Wq, Wk, Wv)
    last_err = None
    for _attempt in range(3):
        try:
            res = run_bass_kernel_spmd(
                nc, in_maps, core_ids=list(range(NCORES)), **spmd_kwargs
            )
            break
        except Exception as e:  # transient NRT_EXEC_UNIT_UNRECOVERABLE faults
            last_err = e
    else:
        raise last_err
    out = np.concatenate([res.results[c]["out"] for c in range(NCORES)], axis=1)
    return out, res


def kernel(x_1, x_2, Wq, Wk, Wv):
    out, _ = run(x_1, x_2, Wq, Wk, Wv)
    return out.astype(np.float32)


# revision 17
# speedup vs baseline: 1.1772x; 1.0514x over previous
"""Cross-attention Bass/Tile kernel for Trainium2, data-parallel over batch on 8 cores.

Problem (hardcoded): x_1 [2048,16,100], x_2 [2048,16,100], Wq/Wk/Wv [100,128], fp32.
  Q = x1 @ Wq; K = x2 @ Wk; V = x2 @ Wv  (per batch)
  out = softmax(Q K^T / sqrt(128)) @ V   -> [2048,16,128]

Sharding: batch dim split 8 ways (2 batches per core). Full inputs in, full output out.

v2 design notes (ACT-engine paced):
  The per-core floor is the scalar/ACT engine: 2*S*S = 8.4M exps at 1 elem/lane/cycle
  @1.2GHz = 54.6us + per-instr bubbles -> ~68-71us for 64 [128,1024] EXPs. PE issue
  work (S^T + PV + prep) is ~65us. So ACT must do NOTHING but the exps, back-to-back:
   - x loaded fp32 via 8 batched DMAs (rearrange, 4 t-tiles each); PE transposes fp32
     directly (1 col/cycle, trace-verified); Pool evicts+casts psum->bf16. No input
     casts on ACT.
   - tail normalize-muls on Pool (tensor_scalar_mul), psum evictions on Pool/DVE.
   - rowsum tree on DVE all-16-bit (fp16 mid levels keep the 2x DVE mode).
   - pending tails emitted at TOP of each chunk: baseline emitted them after the
     next chunk's pair loop, making the last chunk's self-PV matmuls wait on a PSUM
     WAR hazard (6.1us end stall in the trace).
   - EXP activation table preloaded via a dummy [128,1] exp during ramp.
   - one output DMA per chunk ([128,4,128] rearrange), engine-issue cost ~600ns each.

Per-core dataflow (2 batches b0,b1; 4 chunks of 512 s-cols each; pairs g = 2 t-tiles):
  S^T pair [128,1024] fp32 psum (2 matmuls) -> ACT exp*scale -> et bf16 SBUF
  PV of prev chunk (2 accum matmuls) interleaved after each exp; last chunk self-PV.
  rowsum: DVE tree et[8192] -> fp16 -> rowacc bf16 [128,512]
  tail: Pool evict O^T, PE ones-matmuls (denominators), DVE recip, PE transposes,
        Pool scale, sync DMA out.
  Ramp: b0 prep (fp32 transposes + QK projections) interleaved just-in-time into
  chunk 0's pair loop; b1 prep during chunks 1-2.
"""

import sys

sys.path.insert(0, "/opt/trn_rl_repo")

import numpy as np

import concourse.bass as bass
import concourse.tile as tile
from concourse import bacc, mybir
from concourse.bass_utils import run_bass_kernel_spmd
from concourse.masks import make_identity

S = 2048
B = 16
DH = 100
DK = 128
NCORES = 8
BPC = B // NCORES
F32 = mybir.dt.float32
BF16 = mybir.dt.bfloat16
FP16 = mybir.dt.float16
SCALE = 1.0 / float(np.sqrt(np.float32(DK)))

ST = S // 128     # 16 t-tiles of 128
NSC = S // 512    # 4 chunks of 512 per batch
XCOLS = ST * 2 * DH + 32  # [128, 3232]: 16 tiles x 200 cols + pad for b1/k15 window


def _attention_kernel(tc, out, x1, x2, wq, wk, wv):
    nc = tc.nc

    with (
        tc.tile_pool(name="const", bufs=1) as constp,
        tc.tile_pool(name="xn", bufs=2) as xnp,
        tc.tile_pool(name="xT", bufs=4) as xtp,
        tc.tile_pool(name="qk", bufs=4) as qkp,
        tc.tile_pool(name="vp", bufs=2) as vp,
        tc.tile_pool(name="et", bufs=2) as etp,
        tc.tile_pool(name="acc", bufs=2) as accp,
        tc.tile_pool(name="rowb", bufs=2) as rowbp,
        tc.tile_pool(name="rr", bufs=2) as rrp,
        tc.tile_pool(name="osb", bufs=2) as osbp,
        tc.tile_pool(name="osc", bufs=2) as oscp,
        tc.tile_pool(name="ps_st", bufs=2, space="PSUM") as psb,
        tc.tile_pool(name="ps_ot", bufs=2, space="PSUM") as psot,
        tc.tile_pool(name="ps_sc", bufs=2, space="PSUM") as pssc,
    ):
        # ---- identity first (gpsimd) so it doesn't queue behind DMA issues
        ident = constp.tile([128, 128], F32)
        make_identity(nc, ident)
        ident_bf = constp.tile([128, 128], BF16)
        nc.vector.tensor_copy(ident_bf, ident)

        # ---- x loads as bf16 tiles [128, 16x200]. Group 0 of each source is
        # ramp-critical: plain fp32 DMA (sync/scalar HWDGE, split in halves for
        # parallel queues) + DVE cast. Groups 1-3: gpsimd DGE-cast DMAs.
        xn_tiles = {}
        for src_i in (0, 1):
            xn_tiles[src_i] = xnp.tile(
                [128, XCOLS], BF16, tag="xn", name=f"xn{src_i}"
            )

        def x_group_src(src_ap, g, k=4):
            return src_ap[g * 512:(g + 1) * 512, :, :].rearrange(
                "(k p) b d -> p k (b d)", k=k
            )

        stg = {}
        for src_i, src_ap, eng in ((1, x2, nc.sync), (0, x1, nc.scalar)):
            stg[src_i] = constp.tile([128, 800], F32, name=f"stg{src_i}")
            for h in range(2):
                half = src_ap[h * 256:(h + 1) * 256, :, :].rearrange(
                    "(k p) b d -> p k (b d)", k=2
                )
                eng.dma_start(stg[src_i][:, h * 400:(h + 1) * 400], half)
        w_f32s = {}
        for wname, wap in (("wk", wk), ("wq", wq), ("wv", wv)):
            w_f32s[wname] = constp.tile([DH, DK], F32, name=f"{wname}_f32")
            nc.sync.dma_start(w_f32s[wname], wap)
        for g in range(1, 4):
            nc.gpsimd.dma_start(
                xn_tiles[1][:, g * 800:(g + 1) * 800], x_group_src(x2, g)
            )
            nc.gpsimd.dma_start(
                xn_tiles[0][:, g * 800:(g + 1) * 800], x_group_src(x1, g)
            )

        # ---- constants / casts
        ones_bf = constp.tile([128, 1], BF16)
        nc.vector.memset(ones_bf, 1.0)
        w_sbs = {}
        for wname in ("wk", "wq", "wv"):
            w_sb = constp.tile([DH, DK], BF16, name=f"{wname}_sb")
            nc.vector.tensor_copy(w_sb, w_f32s[wname])
            w_sbs[wname] = w_sb
        wq_sb, wk_sb, wv_sb = w_sbs["wq"], w_sbs["wk"], w_sbs["wv"]
        # preload the EXP activation table during ramp (dummy exp)
        dum = constp.tile([128, 1], F32, name="dum")
        nc.vector.memset(dum, 0.0)
        dum_o = constp.tile([128, 1], BF16, name="dum_o")
        nc.scalar.activation(dum_o, dum, mybir.ActivationFunctionType.Exp)
        # group-0 casts fp32 -> bf16 (DVE)
        nc.vector.tensor_copy(xn_tiles[1][:, :800], stg[1])
        nc.vector.tensor_copy(xn_tiles[0][:, :800], stg[0])
        # zero the pad window read by the b=1, k=15 transpose slice
        for src_i in (0, 1):
            nc.gpsimd.memset(xn_tiles[src_i][:, ST * 2 * DH:], 0.0)

        # persistent transposed/projected tensors
        xTs, qTs, kTs, vas = {}, {}, {}, {}
        for src_i in (0, 1):
            for b in range(BPC):
                xTs[(src_i, b)] = xtp.tile(
                    [128, S], BF16, tag="xT", name=f"xT_{src_i}_{b}"
                )
        for b in range(BPC):
            qTs[b] = qkp.tile([DK, S], BF16, tag="qk", name=f"qT_{b}")
            kTs[b] = qkp.tile([DK, S], BF16, tag="qk", name=f"kT_{b}")
            vas[b] = vp.tile([128, S], BF16, tag="v", name=f"vall_{b}")

        def emit_xt_g(src_i, b, g):
            """Transpose 4 bf16 t-tiles on PE, evict psum->SBUF xT on DVE."""
            psq = pssc.tile([128, 512], BF16, tag="sc", name=f"xq_{src_i}_{b}_{g}")
            xn = xn_tiles[src_i]
            for j in range(4):
                tt = g * 4 + j
                c0 = tt * 2 * DH + b * DH
                nc.tensor.transpose(
                    psq[:, j * 128:(j + 1) * 128], xn[:, c0:c0 + 128], ident_bf
                )
            nc.vector.tensor_copy(xTs[(src_i, b)][:, g * 512:(g + 1) * 512], psq)

        def emit_proj_c(dstT, w_sb, xT, b, c, eng=None):
            csl = slice(c * 512, (c + 1) * 512)
            pj = pssc.tile([128, 512], F32, tag="sc", name=f"pj_{b}_{c}")
            nc.tensor.matmul(pj, w_sb, xT[:DH, csl], start=True, stop=True)
            if eng is nc.scalar:
                nc.scalar.copy(dstT[:, csl], pj)
            else:
                nc.vector.tensor_copy(dstT[:, csl], pj)

        def emit_prep_qk(b):
            for src_i in (0, 1):
                for g in range(4):
                    emit_xt_g(src_i, b, g)
            for dstT, w_sb, xT in (
                (qTs[b], wq_sb, xTs[(0, b)]), (kTs[b], wk_sb, xTs[(1, b)])
            ):
                for c in range(NSC):
                    emit_proj_c(dstT, w_sb, xT, b, c)

        def emit_prep_v(b):
            x2T = xTs[(1, b)]
            for g in range(4):
                psv = pssc.tile([128, 512], F32, tag="sc", name=f"vg_{b}_{g}")
                for j in range(4):
                    tt = g * 4 + j
                    nc.tensor.matmul(
                        psv[:, j * 128:(j + 1) * 128],
                        x2T[:DH, tt * 128:(tt + 1) * 128],
                        wv_sb,
                        start=True, stop=True,
                    )
                nc.vector.tensor_copy(vas[b][:, g * 512:(g + 1) * 512], psv)

        def emit_tail_evict(st_):
            """Free the psot buf early: O^T psum -> SBUF on DVE."""
            b, sc, rowacc, otp = st_
            ot_sb = osbp.tile([128, 512], F32, tag="osb", name=f"otsb_{b}_{sc}")
            nc.vector.tensor_copy(ot_sb, otp)
            return ot_sb

        def emit_tail_rest(st_, ot_sb):
            b, sc, rowacc, otp = st_
            rs_all = pssc.tile([128, 4], F32, tag="sc", name=f"rs_{b}_{sc}")
            for si in range(4):
                nc.tensor.matmul(
                    rs_all[:, si:si + 1],
                    rowacc[:, si * 128:(si + 1) * 128], ones_bf,
                    start=True, stop=True,
                )
            rr_all = rrp.tile([128, 4], F32, tag="rr", name=f"rr_{b}_{sc}")
            nc.vector.reciprocal(rr_all, rs_all)
            otr_all = pssc.tile([128, 512], F32, tag="sc", name=f"otr_{b}_{sc}")
            osc_all = oscp.tile([128, 512], F32, tag="osc", name=f"osc_{b}_{sc}")
            for si in range(4):
                scol = slice(si * 128, (si + 1) * 128)
                nc.tensor.transpose(otr_all[:, scol], ot_sb[:, scol], ident)
                nc.vector.tensor_scalar_mul(
                    osc_all[:, scol], otr_all[:, scol], rr_all[:, si:si + 1]
                )
            dst = out[sc * 512:(sc + 1) * 512, b, :].rearrange(
                "(k p) d -> p k d", k=4
            )
            nc.sync.dma_start(dst, osc_all)

        # ---- main loop: 8 chunks; PV of chunk i-1 interleaves into chunk i.
        items = [(b, sc) for b in range(BPC) for sc in range(NSC)]
        prev = None          # (b, sc, et, rowacc, vall)
        pending_tail = None
        otp_self = None
        acch_last = None
        for idx, (b, sc) in enumerate(items):
            if idx == 1:
                emit_prep_qk(1)
            if idx == 2:
                emit_prep_v(1)
            # evict the pending tail's psum first: frees the psot buf before
            # this chunk claims it; the rest of the tail is emitted after
            # pair 1 so a lagging rowsum can't stall the S^T/exp stream
            tail_ot_sb = None
            if pending_tail is not None:
                tail_ot_sb = emit_tail_evict(pending_tail)
            qT, kT = qTs[b], kTs[b]
            ssl = slice(sc * 512, (sc + 1) * 512)
            et = etp.tile([128, ST * 512], BF16, tag="et", name=f"et_{b}_{sc}")
            if prev is not None:
                potp = psot.tile([128, 512], F32, tag="ot",
                                 name=f"ot_{prev[0]}_{prev[1]}")
            is_last = idx == len(items) - 1
            if is_last:
                otp_self = psot.tile([128, 512], F32, tag="ot", name=f"ot_{b}_{sc}")
                acch_last = accp.tile([128, 4096], FP16, tag="acc",
                                      name=f"acch_{b}_{sc}")
            for g in range(ST // 2):
                if g == 2 and pending_tail is not None:
                    emit_tail_rest(pending_tail, tail_ot_sb)
                    pending_tail = None
                if idx == 0:
                    # just-in-time b0 prep interleave
                    if g == 0:
                        emit_xt_g(1, 0, 0)
                        emit_proj_c(kTs[0], wk_sb, xTs[(1, 0)], 0, 0)
                        emit_xt_g(0, 0, 0)
                        emit_proj_c(qTs[0], wq_sb, xTs[(0, 0)], 0, 0,
                                    eng=nc.scalar)
                    elif g in (1, 3, 5):
                        gp = (g + 1) // 2
                        emit_xt_g(1, 0, gp)
                        emit_proj_c(kTs[0], wk_sb, xTs[(1, 0)], 0, gp)
                ps = psb.tile([128, 1024], F32, tag="st", name=f"st_{b}_{sc}_{g}")
                for h in range(2):
                    tt = g * 2 + h
                    nc.tensor.matmul(
                        ps[:, h * 512:(h + 1) * 512],
                        kT[:, tt * 128:(tt + 1) * 128],
                        qT[:, ssl],
                        start=True, stop=True,
                    )
                nc.scalar.activation(
                    et[:, g * 1024:(g + 1) * 1024], ps,
                    mybir.ActivationFunctionType.Exp, scale=SCALE,
                )
                if prev is not None:
                    pb, psc, pet, prow, pvall = prev
                    for h in range(2):
                        ptt = g * 2 + h
                        nc.tensor.matmul(
                            potp,
                            pvall[:, ptt * 128:(ptt + 1) * 128],
                            pet[:, ptt * 512:(ptt + 1) * 512],
                            start=(ptt == 0),
                            stop=(ptt == ST - 1),
                        )
                if is_last:
                    for h in range(2):
                        tt = g * 2 + h
                        nc.tensor.matmul(
                            otp_self,
                            vas[b][:, tt * 128:(tt + 1) * 128],
                            et[:, tt * 512:(tt + 1) * 512],
                            start=(tt == 0),
                            stop=(tt == ST - 1),
                        )
                    # incremental rowsum to shorten the drain
                    if g == 3:
                        nc.vector.tensor_add(
                            acch_last[:, :2048], et[:, :2048], et[:, 2048:4096]
                        )
                    elif g == 5:
                        nc.vector.tensor_add(
                            acch_last[:, :2048], acch_last[:, :2048],
                            et[:, 4096:6144],
                        )
            if idx == 0:
                for gp in range(1, 4):
                    emit_xt_g(0, 0, gp)
                    emit_proj_c(qTs[0], wq_sb, xTs[(0, 0)], 0, gp)
                emit_prep_v(0)
            # rowsum: DVE tree, all 16-bit to keep the 2x perf mode
            rowacc = rowbp.tile([128, 512], BF16, tag="rowb", name=f"row_{b}_{sc}")
            if is_last:
                nc.vector.tensor_add(
                    acch_last[:, :2048], acch_last[:, :2048], et[:, 6144:8192]
                )
                nc.vector.tensor_add(
                    acch_last[:, :1024], acch_last[:, :1024],
                    acch_last[:, 1024:2048],
                )
                nc.vector.tensor_add(
                    rowacc, acch_last[:, :512], acch_last[:, 512:1024]
                )
            else:
                # L1 on DVE (2x mode, 16-bit); L2-L4 on Pool (SBUF-only engine,
                # ~0.5 elem/cycle/lane but otherwise idle in steady state)
                acch = accp.tile([128, 4096], FP16, tag="acc", name=f"acch_{b}_{sc}")
                nc.vector.tensor_add(acch, et[:, :4096], et[:, 4096:])
                nc.gpsimd.tensor_add(acch[:, :2048], acch[:, :2048], acch[:, 2048:])
                nc.gpsimd.tensor_add(acch[:, :1024], acch[:, :1024],
                                     acch[:, 1024:2048])
                nc.gpsimd.tensor_add(rowacc, acch[:, :512], acch[:, 512:1024])
            if prev is not None:
                pending_tail = (prev[0], prev[1], prev[3], potp)
            prev = (b, sc, et, rowacc, vas[b])
        # drain: tails of the final two chunks
        pb, psc, pet, prow, pvall = prev
        if pending_tail is not None:
            osb6 = emit_tail_evict(pending_tail)
            emit_tail_rest(pending_tail, osb6)
        last_t = (pb, psc, prow, otp_self)
        osb7 = emit_tail_evict(last_t)
        emit_tail_rest(last_t, osb7)


_NC_CACHE = None


def _build():
    global _NC_CACHE
    if _NC_CACHE is not None:
        return _NC_CACHE
    nc = bacc.Bacc("TRN2", target_bir_lowering=False, debug=False, num_devices=NCORES)
    x1 = nc.dram_tensor("x_1", (S, BPC, DH), F32, kind="ExternalInput").ap()
    x2 = nc.dram_tensor("x_2", (S, BPC, DH), F32, kind="ExternalInput").ap()
    wq = nc.dram_tensor("Wq", (DH, DK), F32, kind="ExternalInput").ap()
    wk = nc.dram_tensor("Wk", (DH, DK), F32, kind="ExternalInput").ap()
    wv = nc.dram_tensor("Wv", (DH, DK), F32, kind="ExternalInput").ap()
    out = nc.dram_tensor("out", (S, BPC, DK), F32, kind="ExternalOutput").ap()
    with tile.TileContext(nc) as tc:
        _attention_kernel(tc, out, x1, x2, wq, wk, wv)
    nc.compile()
    _NC_CACHE = nc
    return nc


def _in_maps(x_1, x_2, Wq, Wk, Wv):
    maps = []
    for c in range(NCORES):
        bsl = slice(c * BPC, (c + 1) * BPC)
        maps.append({
            "x_1": np.ascontiguousarray(x_1[:, bsl, :], dtype=np.float32),
            "x_2": np.ascontiguousarray(x_2[:, bsl, :], dtype=np.float32),
            "Wq": np.asarray(Wq, dtype=np.float32),
            "Wk": np.asarray(Wk, dtype=np.float32),
            "Wv": np.asarray(Wv, dtype=np.float32),
        })
    return maps


def run(x_1, x_2, Wq, Wk, Wv, **spmd_kwargs):
    nc = _build()
    in_maps = _in_maps(x_1, x_2, Wq, Wk, Wv)
    last_err = None
    for _attempt in range(3):
        try:
            res = run_bass_kernel_spmd(
                nc, in_maps, core_ids=list(range(NCORES)), **spmd_kwargs
            )
            break
        except Exception as e:  # transient NRT_EXEC_UNIT_UNRECOVERABLE faults
            last_err = e
    else:
        raise last_err
    out = np.concatenate([res.results[c]["out"] for c in range(NCORES)], axis=1)
    return out, res


def kernel(x_1, x_2, Wq, Wk, Wv):
    out, _ = run(x_1, x_2, Wq, Wk, Wv)
    return out.astype(np.float32)


# revision 22
# speedup vs baseline: 1.2684x; 1.0775x over previous
"""Cross-attention Bass/Tile kernel for Trainium2, data-parallel over batch on 8 cores.

Problem (hardcoded): x_1 [2048,16,100], x_2 [2048,16,100], Wq/Wk/Wv [100,128], fp32.
  Q = x1 @ Wq; K = x2 @ Wk; V = x2 @ Wv  (per batch)
  out = softmax(Q K^T / sqrt(128)) @ V   -> [2048,16,128]

Sharding: batch dim split 8 ways (2 batches per core). Full inputs in, full output out.

v2 design notes (ACT-engine paced):
  The per-core floor is the scalar/ACT engine: 2*S*S = 8.4M exps at 1 elem/lane/cycle
  @1.2GHz = 54.6us + per-instr bubbles -> ~68-71us for 64 [128,1024] EXPs. PE issue
  work (S^T + PV + prep) is ~65us. So ACT must do NOTHING but the exps, back-to-back:
   - x loaded fp32 via 8 batched DMAs (rearrange, 4 t-tiles each); PE transposes fp32
     directly (1 col/cycle, trace-verified); Pool evicts+casts psum->bf16. No input
     casts on ACT.
   - tail normalize-muls on Pool (tensor_scalar_mul), psum evictions on Pool/DVE.
   - rowsum tree on DVE all-16-bit (fp16 mid levels keep the 2x DVE mode).
   - pending tails emitted at TOP of each chunk: baseline emitted them after the
     next chunk's pair loop, making the last chunk's self-PV matmuls wait on a PSUM
     WAR hazard (6.1us end stall in the trace).
   - EXP activation table preloaded via a dummy [128,1] exp during ramp.
   - one output DMA per chunk ([128,4,128] rearrange), engine-issue cost ~600ns each.

Per-core dataflow (2 batches b0,b1; 4 chunks of 512 s-cols each; pairs g = 2 t-tiles):
  S^T pair [128,1024] fp32 psum (2 matmuls) -> ACT exp*scale -> et bf16 SBUF
  PV of prev chunk (2 accum matmuls) interleaved after each exp; last chunk self-PV.
  rowsum: DVE tree et[8192] -> fp16 -> rowacc bf16 [128,512]
  tail: Pool evict O^T, PE ones-matmuls (denominators), DVE recip, PE transposes,
        Pool scale, sync DMA out.
  Ramp: b0 prep (fp32 transposes + QK projections) interleaved just-in-time into
  chunk 0's pair loop; b1 prep during chunks 1-2.
"""

import sys

sys.path.insert(0, "/opt/trn_rl_repo")

import numpy as np

import concourse.bass as bass
import concourse.tile as tile
from concourse import bacc, mybir
from concourse.bass_utils import run_bass_kernel_spmd
from concourse.masks import make_identity

S = 2048
B = 16
DH = 100
DK = 128
NCORES = 8
BPC = B // NCORES
F32 = mybir.dt.float32
BF16 = mybir.dt.bfloat16
FP16 = mybir.dt.float16
SCALE = 1.0 / float(np.sqrt(np.float32(DK)))

ST = S // 128     # 16 t-tiles of 128
NSC = S // 512    # 4 chunks of 512 per batch
XCOLS = ST * 2 * DH + 32  # [128, 3232]: 16 tiles x 200 cols + pad for b1/k15 window


def _attention_kernel(tc, out, x1, x2, wq, wk, wv):
    nc = tc.nc

    with (
        tc.tile_pool(name="const", bufs=1) as constp,
        tc.tile_pool(name="xn", bufs=2) as xnp,
        tc.tile_pool(name="xT", bufs=4) as xtp,
        tc.tile_pool(name="qk", bufs=4) as qkp,
        tc.tile_pool(name="vp", bufs=2) as vp,
        tc.tile_pool(name="et", bufs=2) as etp,
        tc.tile_pool(name="acc", bufs=2) as accp,
        tc.tile_pool(name="rowb", bufs=2) as rowbp,
        tc.tile_pool(name="rr", bufs=2) as rrp,
        tc.tile_pool(name="osb", bufs=2) as osbp,
        tc.tile_pool(name="osc", bufs=2) as oscp,
        tc.tile_pool(name="ps_st", bufs=2, space="PSUM") as psb,
        tc.tile_pool(name="ps_ot", bufs=2, space="PSUM") as psot,
        tc.tile_pool(name="ps_sc", bufs=2, space="PSUM") as pssc,
    ):
        # ---- identity first (gpsimd) so it doesn't queue behind DMA issues
        ident = constp.tile([128, 128], F32)
        make_identity(nc, ident)
        ident_bf = constp.tile([128, 128], BF16)
        nc.vector.tensor_copy(ident_bf, ident)

        # ---- x loads as bf16 tiles [128, 16x200]. Group 0 of each source is
        # ramp-critical: plain fp32 DMA (sync/scalar HWDGE, split in halves for
        # parallel queues) + DVE cast. Groups 1-3: gpsimd DGE-cast DMAs.
        xn_tiles = {}
        for src_i in (0, 1):
            xn_tiles[src_i] = xnp.tile(
                [128, XCOLS], BF16, tag="xn", name=f"xn{src_i}"
            )

        def x_group_src(src_ap, g, k=4):
            return src_ap[g * 512:(g + 1) * 512, :, :].rearrange(
                "(k p) b d -> p k (b d)", k=k
            )

        # weights first on sync (small transfers, needed early for projections)
        w_f32s = {}
        for wname, wap in (("wk", wk), ("wq", wq), ("wv", wv)):
            w_f32s[wname] = constp.tile([DH, DK], F32, name=f"{wname}_f32")
            nc.sync.dma_start(w_f32s[wname], wap)
        # group-0 halves split across sync/scalar DMA queues for parallelism
        stg = {}
        for src_i, src_ap in ((1, x2), (0, x1)):
            stg[src_i] = constp.tile([128, 800], F32, name=f"stg{src_i}")
            for h, eng in ((0, nc.sync), (1, nc.scalar)):
                half = src_ap[h * 256:(h + 1) * 256, :, :].rearrange(
                    "(k p) b d -> p k (b d)", k=2
                )
                eng.dma_start(stg[src_i][:, h * 400:(h + 1) * 400], half)
        for g in range(1, 4):
            nc.gpsimd.dma_start(
                xn_tiles[1][:, g * 800:(g + 1) * 800], x_group_src(x2, g)
            )
            nc.gpsimd.dma_start(
                xn_tiles[0][:, g * 800:(g + 1) * 800], x_group_src(x1, g)
            )

        # ---- constants / casts
        ones_bf = constp.tile([128, 1], BF16)
        nc.vector.memset(ones_bf, 1.0)
        w_sbs = {}
        for wname in ("wk", "wq", "wv"):
            w_sb = constp.tile([DH, DK], BF16, name=f"{wname}_sb")
            nc.vector.tensor_copy(w_sb, w_f32s[wname])
            w_sbs[wname] = w_sb
        wq_sb, wk_sb, wv_sb = w_sbs["wq"], w_sbs["wk"], w_sbs["wv"]
        # preload the EXP activation table during ramp (dummy exp)
        dum = constp.tile([128, 1], F32, name="dum")
        nc.vector.memset(dum, 0.0)
        dum_o = constp.tile([128, 1], BF16, name="dum_o")
        nc.scalar.activation(dum_o, dum, mybir.ActivationFunctionType.Exp)
        # group-0 casts fp32 -> bf16 (DVE)
        nc.vector.tensor_copy(xn_tiles[1][:, :800], stg[1])
        nc.vector.tensor_copy(xn_tiles[0][:, :800], stg[0])
        # zero the pad window read by the b=1, k=15 transpose slice
        for src_i in (0, 1):
            nc.gpsimd.memset(xn_tiles[src_i][:, ST * 2 * DH:], 0.0)

        # persistent transposed/projected tensors
        xTs, qTs, kTs, vas = {}, {}, {}, {}
        for src_i in (0, 1):
            for b in range(BPC):
                xTs[(src_i, b)] = xtp.tile(
                    [128, S], BF16, tag="xT", name=f"xT_{src_i}_{b}"
                )
        for b in range(BPC):
            qTs[b] = qkp.tile([DK, S], BF16, tag="qk", name=f"qT_{b}")
            kTs[b] = qkp.tile([DK, S], BF16, tag="qk", name=f"kT_{b}")
            vas[b] = vp.tile([128, S], BF16, tag="v", name=f"vall_{b}")

        def emit_xt_g(src_i, b, g):
            """Transpose 4 bf16 t-tiles on PE, evict psum->SBUF xT on DVE."""
            psq = pssc.tile([128, 512], BF16, tag="sc", name=f"xq_{src_i}_{b}_{g}")
            xn = xn_tiles[src_i]
            for j in range(4):
                tt = g * 4 + j
                c0 = tt * 2 * DH + b * DH
                nc.tensor.transpose(
                    psq[:, j * 128:(j + 1) * 128], xn[:, c0:c0 + 128], ident_bf
                )
            nc.vector.tensor_copy(xTs[(src_i, b)][:, g * 512:(g + 1) * 512], psq)

        def emit_proj_c(dstT, w_sb, xT, b, c, eng=None):
            csl = slice(c * 512, (c + 1) * 512)
            pj = pssc.tile([128, 512], F32, tag="sc", name=f"pj_{b}_{c}")
            nc.tensor.matmul(pj, w_sb, xT[:DH, csl], start=True, stop=True)
            if eng is nc.scalar:
                nc.scalar.copy(dstT[:, csl], pj)
            else:
                nc.vector.tensor_copy(dstT[:, csl], pj)

        def emit_prep_qk(b):
            for src_i in (0, 1):
                for g in range(4):
                    emit_xt_g(src_i, b, g)
            for dstT, w_sb, xT in (
                (qTs[b], wq_sb, xTs[(0, b)]), (kTs[b], wk_sb, xTs[(1, b)])
            ):
                for c in range(NSC):
                    emit_proj_c(dstT, w_sb, xT, b, c)

        def emit_prep_v(b):
            x2T = xTs[(1, b)]
            for g in range(4):
                psv = pssc.tile([128, 512], F32, tag="sc", name=f"vg_{b}_{g}")
                for j in range(4):
                    tt = g * 4 + j
                    nc.tensor.matmul(
                        psv[:, j * 128:(j + 1) * 128],
                        x2T[:DH, tt * 128:(tt + 1) * 128],
                        wv_sb,
                        start=True, stop=True,
                    )
                nc.vector.tensor_copy(vas[b][:, g * 512:(g + 1) * 512], psv)

        def emit_tail_evict(st_):
            """Free the psot buf early: O^T psum -> SBUF (bf16) on DVE."""
            b, sc, rowacc, otp = st_
            ot_sb = osbp.tile([128, 512], BF16, tag="osb", name=f"otsb_{b}_{sc}")
            nc.vector.tensor_copy(ot_sb, otp)
            return ot_sb

        def emit_tail_rest(st_, ot_sb):
            b, sc, rowacc, otp = st_
            rs_all = pssc.tile([128, 4], F32, tag="sc", name=f"rs_{b}_{sc}")
            for si in range(4):
                nc.tensor.matmul(
                    rs_all[:, si:si + 1],
                    rowacc[:, si * 128:(si + 1) * 128], ones_bf,
                    start=True, stop=True,
                )
            rr_all = rrp.tile([128, 4], F32, tag="rr", name=f"rr_{b}_{sc}")
            nc.vector.reciprocal(rr_all, rs_all)
            otr_all = pssc.tile([128, 512], BF16, tag="sc", name=f"otr_{b}_{sc}")
            osc_all = oscp.tile([128, 512], F32, tag="osc", name=f"osc_{b}_{sc}")
            for si in range(4):
                scol = slice(si * 128, (si + 1) * 128)
                nc.tensor.transpose(otr_all[:, scol], ot_sb[:, scol], ident_bf)
                nc.vector.tensor_scalar_mul(
                    osc_all[:, scol], otr_all[:, scol], rr_all[:, si:si + 1]
                )
            dst = out[sc * 512:(sc + 1) * 512, b, :].rearrange(
                "(k p) d -> p k d", k=4
            )
            nc.sync.dma_start(dst, osc_all)

        # ---- main loop: 8 chunks; PV of chunk i-1 interleaves into chunk i.
        items = [(b, sc) for b in range(BPC) for sc in range(NSC)]
        prev = None          # (b, sc, et, rowacc, vall)
        pending_tail = None
        otp_self = None
        acch_last = None
        for idx, (b, sc) in enumerate(items):
            # b1 prep spread over chunks 1-3 to respect DVE per-chunk slack
            if idx == 1:
                for g in range(4):
                    emit_xt_g(1, 1, g)
                for c in range(NSC):
                    emit_proj_c(kTs[1], wk_sb, xTs[(1, 1)], 1, c)
            if idx == 2:
                for g in range(4):
                    emit_xt_g(0, 1, g)
                for c in range(NSC):
                    emit_proj_c(qTs[1], wq_sb, xTs[(0, 1)], 1, c)
            if idx == 3:
                emit_prep_v(1)
            # evict the pending tail's psum first: frees the psot buf before
            # this chunk claims it; the rest of the tail is emitted after
            # pair 1 so a lagging rowsum can't stall the S^T/exp stream
            tail_ot_sb = None
            if pending_tail is not None:
                tail_ot_sb = emit_tail_evict(pending_tail)
            qT, kT = qTs[b], kTs[b]
            ssl = slice(sc * 512, (sc + 1) * 512)
            et = etp.tile([128, ST * 512], BF16, tag="et", name=f"et_{b}_{sc}")
            if prev is not None:
                potp = psot.tile([128, 512], F32, tag="ot",
                                 name=f"ot_{prev[0]}_{prev[1]}")
            is_last = idx == len(items) - 1
            if is_last:
                otp_self = psot.tile([128, 512], F32, tag="ot", name=f"ot_{b}_{sc}")
                acch_last = accp.tile([128, 4096], FP16, tag="acc",
                                      name=f"acch_{b}_{sc}")
            for g in range(ST // 2):
                if g == 6 and pending_tail is not None:
                    emit_tail_rest(pending_tail, tail_ot_sb)
                    pending_tail = None
                if idx == 0:
                    # just-in-time b0 prep interleave
                    if g == 0:
                        emit_xt_g(1, 0, 0)
                        emit_proj_c(kTs[0], wk_sb, xTs[(1, 0)], 0, 0)
                        emit_xt_g(0, 0, 0)
                        emit_proj_c(qTs[0], wq_sb, xTs[(0, 0)], 0, 0,
                                    eng=nc.scalar)
                    elif g in (1, 3, 5):
                        gp = (g + 1) // 2
                        emit_xt_g(1, 0, gp)
                        emit_proj_c(kTs[0], wk_sb, xTs[(1, 0)], 0, gp)
                ps = psb.tile([128, 1024], F32, tag="st", name=f"st_{b}_{sc}_{g}")
                for h in range(2):
                    tt = g * 2 + h
                    nc.tensor.matmul(
                        ps[:, h * 512:(h + 1) * 512],
                        kT[:, tt * 128:(tt + 1) * 128],
                        qT[:, ssl],
                        start=True, stop=True,
                    )
                nc.scalar.activation(
                    et[:, g * 1024:(g + 1) * 1024], ps,
                    mybir.ActivationFunctionType.Exp, scale=SCALE,
                )
                if prev is not None:
                    pb, psc, pet, prow, pvall = prev
                    for h in range(2):
                        ptt = g * 2 + h
                        nc.tensor.matmul(
                            potp,
                            pvall[:, ptt * 128:(ptt + 1) * 128],
                            pet[:, ptt * 512:(ptt + 1) * 512],
                            start=(ptt == 0),
                            stop=(ptt == ST - 1),
                        )
                if is_last:
                    for h in range(2):
                        tt = g * 2 + h
                        nc.tensor.matmul(
                            otp_self,
                            vas[b][:, tt * 128:(tt + 1) * 128],
                            et[:, tt * 512:(tt + 1) * 512],
                            start=(tt == 0),
                            stop=(tt == ST - 1),
                        )
                    # incremental rowsum to shorten the drain
                    if g == 3:
                        nc.vector.tensor_add(
                            acch_last[:, :2048], et[:, :2048], et[:, 2048:4096]
                        )
                    elif g == 5:
                        nc.vector.tensor_add(
                            acch_last[:, :2048], acch_last[:, :2048],
                            et[:, 4096:6144],
                        )
            if idx == 0:
                for gp in range(1, 4):
                    emit_xt_g(0, 0, gp)
                    emit_proj_c(qTs[0], wq_sb, xTs[(0, 0)], 0, gp)
                emit_prep_v(0)
            # rowsum: DVE tree, all 16-bit to keep the 2x perf mode
            rowacc = rowbp.tile([128, 512], BF16, tag="rowb", name=f"row_{b}_{sc}")
            if is_last:
                nc.vector.tensor_add(
                    acch_last[:, :2048], acch_last[:, :2048], et[:, 6144:8192]
                )
                nc.vector.tensor_add(
                    acch_last[:, :1024], acch_last[:, :1024],
                    acch_last[:, 1024:2048],
                )
                nc.vector.tensor_add(
                    rowacc, acch_last[:, :512], acch_last[:, 512:1024]
                )
            else:
                # L1 on DVE (2x mode, 16-bit); L2-L4 on Pool (SBUF-only engine,
                # ~0.5 elem/cycle/lane but otherwise idle in steady state)
                acch = accp.tile([128, 4096], FP16, tag="acc", name=f"acch_{b}_{sc}")
                nc.vector.tensor_add(acch, et[:, :4096], et[:, 4096:])
                nc.gpsimd.tensor_add(acch[:, :2048], acch[:, :2048], acch[:, 2048:])
                nc.gpsimd.tensor_add(acch[:, :1024], acch[:, :1024],
                                     acch[:, 1024:2048])
                nc.gpsimd.tensor_add(rowacc, acch[:, :512], acch[:, 512:1024])
            if prev is not None:
                pending_tail = (prev[0], prev[1], prev[3], potp)
            prev = (b, sc, et, rowacc, vas[b])
        # drain: tails of the final two chunks
        pb, psc, pet, prow, pvall = prev
        if pending_tail is not None:
            osb6 = emit_tail_evict(pending_tail)
            emit_tail_rest(pending_tail, osb6)
        last_t = (pb, psc, prow, otp_self)
        osb7 = emit_tail_evict(last_t)
        emit_tail_rest(last_t, osb7)


_NC_CACHE = None


def _build():
    global _NC_CACHE
    if _NC_CACHE is not None:
        return _NC_CACHE
    nc = bacc.Bacc("TRN2", target_bir_lowering=False, debug=False, num_devices=NCORES)
    x1 = nc.dram_tensor("x_1", (S, BPC, DH), F32, kind="ExternalInput").ap()
    x2 = nc.dram_tensor("x_2", (S, BPC, DH), F32, kind="ExternalInput").ap()
    wq = nc.dram_tensor("Wq", (DH, DK), F32, kind="ExternalInput").ap()
    wk = nc.dram_tensor("Wk", (DH, DK), F32, kind="ExternalInput").ap()
    wv = nc.dram_tensor("Wv", (DH, DK), F32, kind="ExternalInput").ap()
    out = nc.dram_tensor("out", (S, BPC, DK), F32, kind="ExternalOutput").ap()
    with tile.TileContext(nc) as tc:
        _attention_kernel(tc, out, x1, x2, wq, wk, wv)
    nc.compile()
    _NC_CACHE = nc
    return nc


def _in_maps(x_1, x_2, Wq, Wk, Wv):
    maps = []
    for c in range(NCORES):
        bsl = slice(c * BPC, (c + 1) * BPC)
        maps.append({
            "x_1": np.ascontiguousarray(x_1[:, bsl, :], dtype=np.float32),
            "x_2": np.ascontiguousarray(x_2[:, bsl, :], dtype=np.float32),
            "Wq": np.asarray(Wq, dtype=np.float32),
            "Wk": np.asarray(Wk, dtype=np.float32),
            "Wv": np.asarray(Wv, dtype=np.float32),
        })
    return maps


def run(x_1, x_2, Wq, Wk, Wv, **spmd_kwargs):
    nc = _build()
    in_maps = _in_maps(x_1, x_2, Wq, Wk, Wv)
    last_err = None
    for _attempt in range(3):
        try:
            res = run_bass_kernel_spmd(
                nc, in_maps, core_ids=list(range(NCORES)), **spmd_kwargs
            )
            break
        except Exception as e:  # transient NRT_EXEC_UNIT_UNRECOVERABLE faults
            last_err = e
    else:
        raise last_err
    out = np.concatenate([res.results[c]["out"] for c in range(NCORES)], axis=1)
    return out, res


def kernel(x_1, x_2, Wq, Wk, Wv):
    out, _ = run(x_1, x_2, Wq, Wk, Wv)
    return out.astype(np.float32)


# revision 33
# speedup vs baseline: 1.2988x; 1.0240x over previous
"""Cross-attention Bass/Tile kernel for Trainium2, data-parallel over batch on 8 cores.

Problem (hardcoded): x_1 [2048,16,100], x_2 [2048,16,100], Wq/Wk/Wv [100,128], fp32.
  Q = x1 @ Wq; K = x2 @ Wk; V = x2 @ Wv  (per batch)
  out = softmax(Q K^T / sqrt(128)) @ V   -> [2048,16,128]

Sharding: batch dim split 8 ways (2 batches per core). Full inputs in, full output out.

v2 design notes (ACT-engine paced):
  The per-core floor is the scalar/ACT engine: 2*S*S = 8.4M exps at 1 elem/lane/cycle
  @1.2GHz = 54.6us + per-instr bubbles -> ~68-71us for 64 [128,1024] EXPs. PE issue
  work (S^T + PV + prep) is ~65us. So ACT must do NOTHING but the exps, back-to-back:
   - x loaded fp32 via 8 batched DMAs (rearrange, 4 t-tiles each); PE transposes fp32
     directly (1 col/cycle, trace-verified); Pool evicts+casts psum->bf16. No input
     casts on ACT.
   - tail normalize-muls on Pool (tensor_scalar_mul), psum evictions on Pool/DVE.
   - rowsum tree on DVE all-16-bit (fp16 mid levels keep the 2x DVE mode).
   - pending tails emitted at TOP of each chunk: baseline emitted them after the
     next chunk's pair loop, making the last chunk's self-PV matmuls wait on a PSUM
     WAR hazard (6.1us end stall in the trace).
   - EXP activation table preloaded via a dummy [128,1] exp during ramp.
   - one output DMA per chunk ([128,4,128] rearrange), engine-issue cost ~600ns each.

Per-core dataflow (2 batches b0,b1; 4 chunks of 512 s-cols each; pairs g = 2 t-tiles):
  S^T pair [128,1024] fp32 psum (2 matmuls) -> ACT exp*scale -> et bf16 SBUF
  PV of prev chunk (2 accum matmuls) interleaved after each exp; last chunk self-PV.
  rowsum: DVE tree et[8192] -> fp16 -> rowacc bf16 [128,512]
  tail: Pool evict O^T, PE ones-matmuls (denominators), DVE recip, PE transposes,
        Pool scale, sync DMA out.
  Ramp: b0 prep (fp32 transposes + QK projections) interleaved just-in-time into
  chunk 0's pair loop; b1 prep during chunks 1-2.
"""

import sys

sys.path.insert(0, "/opt/trn_rl_repo")

import numpy as np

import concourse.bass as bass
import concourse.tile as tile
from concourse import bacc, mybir
from concourse.bass_utils import run_bass_kernel_spmd
from concourse.masks import make_identity

S = 2048
B = 16
DH = 100
DK = 128
NCORES = 8
BPC = B // NCORES
F32 = mybir.dt.float32
BF16 = mybir.dt.bfloat16
FP16 = mybir.dt.float16
SCALE = 1.0 / float(np.sqrt(np.float32(DK)))

ST = S // 128     # 16 t-tiles of 128
NSC = S // 512    # 4 chunks of 512 per batch
XCOLS = ST * 2 * DH + 32  # [128, 3232]: 16 tiles x 200 cols + pad for b1/k15 window


def _attention_kernel(tc, out, x1, x2, wq, wk, wv):
    nc = tc.nc

    with (
        tc.tile_pool(name="const", bufs=1) as constp,
        tc.tile_pool(name="xn", bufs=2) as xnp,
        tc.tile_pool(name="xT", bufs=4) as xtp,
        tc.tile_pool(name="qk", bufs=4) as qkp,
        tc.tile_pool(name="vp", bufs=2) as vp,
        tc.tile_pool(name="et", bufs=2) as etp,
        tc.tile_pool(name="acc", bufs=2) as accp,
        tc.tile_pool(name="rowb", bufs=2) as rowbp,
        tc.tile_pool(name="rr", bufs=2) as rrp,
        tc.tile_pool(name="osb", bufs=2) as osbp,
        tc.tile_pool(name="osc", bufs=2) as oscp,
        tc.tile_pool(name="ps_st", bufs=2, space="PSUM") as psb,
        tc.tile_pool(name="ps_ot", bufs=2, space="PSUM") as psot,
        tc.tile_pool(name="ps_sc", bufs=2, space="PSUM") as pssc,
    ):
        # ---- identity first (gpsimd) so it doesn't queue behind DMA issues
        ident = constp.tile([128, 128], F32)
        make_identity(nc, ident)
        ident_bf = constp.tile([128, 128], BF16)
        nc.vector.tensor_copy(ident_bf, ident)

        # ---- x loads as bf16 tiles [128, 16x200]. DMA is packet-rate limited
        # (~800B packets, ~40-60GB/s per queue stream), so chunk-0's working set
        # (all of x2 + x1 g0) is scheduled by deadline across all three DMA
        # queues (sync/scalar HWDGE fp32 + DVE cast; gpsimd DGE casts inline).
        xn_tiles = {}
        for src_i in (0, 1):
            xn_tiles[src_i] = xnp.tile(
                [128, XCOLS], BF16, tag="xn", name=f"xn{src_i}"
            )

        def x_pair_src(src_ap, p):
            return src_ap[p * 256:(p + 1) * 256, :, :].rearrange(
                "(t q) b d -> q t (b d)", t=2
            )

        def x_group_src(src_ap, g):
            return src_ap[g * 512:(g + 1) * 512, :, :].rearrange(
                "(k p) b d -> p k (b d)", k=4
            )

        w_f32s = {}
        for wname, wap in (("wk", wk), ("wq", wq), ("wv", wv)):
            w_f32s[wname] = constp.tile([DH, DK], F32, name=f"{wname}_f32")
        # sync queue: wk, then x2 pairs 01 / 67 / CD (fp32 staging)
        # scalar queue: wq, then x2 pairs 23 / 89 / EF
        # gpsimd queue: x1 pairs 01/23 (bf16 cast), x2 pairs 45/AB, wv, x1 g1-3
        stg_x2 = constp.tile([128, 2400], F32, name="stg_x2")
        STAGED = {0: 0, 1: 400, 3: 800, 4: 1200, 6: 1600, 7: 2000}
        nc.sync.dma_start(w_f32s["wk"], wk)
        nc.scalar.dma_start(w_f32s["wq"], wq)
        nc.gpsimd.dma_start(xn_tiles[0][:, 0:400], x_pair_src(x1, 0))
        sync_pairs = [0, 3, 6]
        scalar_pairs = [1, 4, 7]
        nc.sync.dma_start(stg_x2[:, 0:400], x_pair_src(x2, 0))
        nc.scalar.dma_start(stg_x2[:, 400:800], x_pair_src(x2, 1))
        nc.gpsimd.dma_start(xn_tiles[0][:, 400:800], x_pair_src(x1, 1))
        nc.sync.dma_start(stg_x2[:, 800:1200], x_pair_src(x2, 3))
        nc.scalar.dma_start(stg_x2[:, 1200:1600], x_pair_src(x2, 4))
        nc.gpsimd.dma_start(xn_tiles[1][:, 800:1200], x_pair_src(x2, 2))
        nc.sync.dma_start(stg_x2[:, 1600:2000], x_pair_src(x2, 6))
        nc.scalar.dma_start(stg_x2[:, 2000:2400], x_pair_src(x2, 7))
        nc.gpsimd.dma_start(xn_tiles[1][:, 2000:2400], x_pair_src(x2, 5))
        nc.gpsimd.dma_start(w_f32s["wv"], wv)
        for g in range(1, 4):
            nc.gpsimd.dma_start(
                xn_tiles[0][:, g * 800:(g + 1) * 800], x_group_src(x1, g)
            )

        # ---- constants / casts
        ones_bf = constp.tile([128, 1], BF16)
        nc.vector.memset(ones_bf, 1.0)
        w_sbs = {}
        for wname in ("wk", "wq", "wv"):
            w_sb = constp.tile([DH, DK], BF16, name=f"{wname}_sb")
            nc.vector.tensor_copy(w_sb, w_f32s[wname])
            w_sbs[wname] = w_sb
        wq_sb, wk_sb, wv_sb = w_sbs["wq"], w_sbs["wk"], w_sbs["wv"]
        # preload the EXP activation table during ramp (dummy exp)
        dum = constp.tile([128, 1], F32, name="dum")
        nc.vector.memset(dum, 0.0)
        dum_o = constp.tile([128, 1], BF16, name="dum_o")
        nc.scalar.activation(dum_o, dum, mybir.ActivationFunctionType.Exp)
        # staged x2 pair casts fp32 -> bf16 (DVE), in deadline order
        for p, off in STAGED.items():
            nc.vector.tensor_copy(
                xn_tiles[1][:, p * 400:(p + 1) * 400], stg_x2[:, off:off + 400]
            )
        # zero the pad window read by the b=1, k=15 transpose slice
        for src_i in (0, 1):
            nc.gpsimd.memset(xn_tiles[src_i][:, ST * 2 * DH:], 0.0)

        # persistent transposed/projected tensors
        xTs, qTs, kTs, vas = {}, {}, {}, {}
        for src_i in (0, 1):
            for b in range(BPC):
                xTs[(src_i, b)] = xtp.tile(
                    [128, S], BF16, tag="xT", name=f"xT_{src_i}_{b}"
                )
        for b in range(BPC):
            qTs[b] = qkp.tile([DK, S], BF16, tag="qk", name=f"qT_{b}")
            kTs[b] = qkp.tile([DK, S], BF16, tag="qk", name=f"kT_{b}")
            vas[b] = vp.tile([128, S], BF16, tag="v", name=f"vall_{b}")

        def emit_xt(src_i, b, t0, nt):
            """Transpose nt bf16 t-tiles on PE, evict psum->SBUF xT on DVE."""
            psq = pssc.tile([128, nt * 128], BF16, tag="sc",
                            name=f"xq_{src_i}_{b}_{t0}")
            xn = xn_tiles[src_i]
            for j in range(nt):
                tt = t0 + j
                c0 = tt * 2 * DH + b * DH
                nc.tensor.transpose(
                    psq[:, j * 128:(j + 1) * 128], xn[:, c0:c0 + 128], ident_bf
                )
            nc.vector.tensor_copy(
                xTs[(src_i, b)][:, t0 * 128:(t0 + nt) * 128], psq
            )

        def emit_proj(dstT, w_sb, xT, b, c0, ncols, eng=None):
            csl = slice(c0, c0 + ncols)
            pj = pssc.tile([128, ncols], F32, tag="sc", name=f"pj_{b}_{c0}")
            nc.tensor.matmul(pj, w_sb, xT[:DH, csl], start=True, stop=True)
            if eng is nc.scalar:
                nc.scalar.copy(dstT[:, csl], pj)
            else:
                nc.vector.tensor_copy(dstT[:, csl], pj)

        def emit_prep_v(b):
            x2T = xTs[(1, b)]
            for g in range(4):
                psv = pssc.tile([128, 512], F32, tag="sc", name=f"vg_{b}_{g}")
                for j in range(4):
                    tt = g * 4 + j
                    nc.tensor.matmul(
                        psv[:, j * 128:(j + 1) * 128],
                        x2T[:DH, tt * 128:(tt + 1) * 128],
                        wv_sb,
                        start=True, stop=True,
                    )
                nc.vector.tensor_copy(vas[b][:, g * 512:(g + 1) * 512], psv)

        def emit_tail_evict(st_):
            """Free the psot buf early: O^T psum -> SBUF (bf16) on DVE."""
            b, sc, rowacc, otp = st_
            ot_sb = osbp.tile([128, 512], BF16, tag="osb", name=f"otsb_{b}_{sc}")
            nc.vector.tensor_copy(ot_sb, otp)
            return ot_sb

        def emit_tail_rest(st_, ot_sb, dma_engs=(nc.sync,)):
            b, sc, rowacc, otp = st_
            rs_all = pssc.tile([128, 4], F32, tag="sc", name=f"rs_{b}_{sc}")
            for si in range(4):
                nc.tensor.matmul(
                    rs_all[:, si:si + 1],
                    rowacc[:, si * 128:(si + 1) * 128], ones_bf,
                    start=True, stop=True,
                )
            rr_all = rrp.tile([128, 4], F32, tag="rr", name=f"rr_{b}_{sc}")
            nc.vector.reciprocal(rr_all, rs_all)
            otr_all = pssc.tile([128, 512], BF16, tag="sc", name=f"otr_{b}_{sc}")
            osc_all = oscp.tile([128, 512], F32, tag="osc", name=f"osc_{b}_{sc}")
            for si in range(4):
                scol = slice(si * 128, (si + 1) * 128)
                nc.tensor.transpose(otr_all[:, scol], ot_sb[:, scol], ident_bf)
                nc.vector.tensor_scalar_mul(
                    osc_all[:, scol], otr_all[:, scol], rr_all[:, si:si + 1]
                )
            # out DMAs are 512B-packet limited (~25GB/s per queue stream):
            # spread chunks across queues so the backlog drains in parallel
            n = len(dma_engs)
            kn = 4 // n
            for i, eng in enumerate(dma_engs):
                s0 = sc * 512 + i * kn * 128
                dst = out[s0:s0 + kn * 128, b, :].rearrange(
                    "(k p) d -> p k d", k=kn
                )
                eng.dma_start(
                    dst, osc_all[:, i * kn * 128:(i + 1) * kn * 128]
                )

        # ---- main loop: 8 chunks; PV of chunk i-1 interleaves into chunk i.
        items = [(b, sc) for b in range(BPC) for sc in range(NSC)]
        prev = None          # (b, sc, et, rowacc, vall)
        pending_tail = None
        otp_self = None
        acch_last = None
        for idx, (b, sc) in enumerate(items):
            # b1 prep spread over chunks 2-4 to respect DVE per-chunk slack
            # (all x tiles are resident by then; b1 is first needed at idx 4)
            if idx == 2:
                for g in range(4):
                    emit_xt(1, 1, g * 4, 4)
                for c in range(NSC):
                    emit_proj(kTs[1], wk_sb, xTs[(1, 1)], 1, c * 512, 512)
            if idx == 3:
                for g in range(4):
                    emit_xt(0, 1, g * 4, 4)
                for c in range(NSC):
                    emit_proj(qTs[1], wq_sb, xTs[(0, 1)], 1, c * 512, 512)
            if idx == 4:
                emit_prep_v(1)
            # evict the pending tail's psum first: frees the psot buf before
            # this chunk claims it; the rest of the tail is emitted after
            # pair 1 so a lagging rowsum can't stall the S^T/exp stream
            tail_ot_sb = None
            if pending_tail is not None:
                tail_ot_sb = emit_tail_evict(pending_tail)
            qT, kT = qTs[b], kTs[b]
            ssl = slice(sc * 512, (sc + 1) * 512)
            et = etp.tile([128, ST * 512], BF16, tag="et", name=f"et_{b}_{sc}")
            if prev is not None:
                potp = psot.tile([128, 512], F32, tag="ot",
                                 name=f"ot_{prev[0]}_{prev[1]}")
            is_last = idx == len(items) - 1
            if is_last:
                otp_self = psot.tile([128, 512], F32, tag="ot", name=f"ot_{b}_{sc}")
                acch_last = accp.tile([128, 4096], FP16, tag="acc",
                                      name=f"acch_{b}_{sc}")
            for g in range(ST // 2):
                if g == 6 and pending_tail is not None:
                    tb, tsc = pending_tail[0], pending_tail[1]
                    eng = nc.sync if (tb * NSC + tsc) % 2 == 0 else nc.gpsimd
                    emit_tail_rest(pending_tail, tail_ot_sb, dma_engs=(eng,))
                    pending_tail = None
                if idx == 0:
                    # just-in-time b0 prep at tile-pair granularity: pair g's
                    # kT columns are emitted ~2 iterations ahead, matching the
                    # deadline-scheduled DMA arrivals
                    if g == 0:
                        emit_xt(1, 0, 0, 2)
                        emit_proj(kTs[0], wk_sb, xTs[(1, 0)], 0, 0, 256)
                        emit_xt(1, 0, 2, 2)
                        emit_proj(kTs[0], wk_sb, xTs[(1, 0)], 0, 256, 256)
                        emit_xt(0, 0, 0, 2)
                        emit_xt(0, 0, 2, 2)
                        emit_proj(qTs[0], wq_sb, xTs[(0, 0)], 0, 0, 512,
                                  eng=nc.scalar)
                    if g <= 5:
                        pr = g + 2
                        emit_xt(1, 0, pr * 2, 2)
                        emit_proj(kTs[0], wk_sb, xTs[(1, 0)], 0, pr * 256, 256)
                ps = psb.tile([128, 1024], F32, tag="st", name=f"st_{b}_{sc}_{g}")
                for h in range(2):
                    tt = g * 2 + h
                    nc.tensor.matmul(
                        ps[:, h * 512:(h + 1) * 512],
                        kT[:, tt * 128:(tt + 1) * 128],
                        qT[:, ssl],
                        start=True, stop=True,
                    )
                nc.scalar.activation(
                    et[:, g * 1024:(g + 1) * 1024], ps,
                    mybir.ActivationFunctionType.Exp, scale=SCALE,
                )
                if prev is not None:
                    pb, psc, pet, prow, pvall = prev
                    for h in range(2):
                        ptt = g * 2 + h
                        nc.tensor.matmul(
                            potp,
                            pvall[:, ptt * 128:(ptt + 1) * 128],
                            pet[:, ptt * 512:(ptt + 1) * 512],
                            start=(ptt == 0),
                            stop=(ptt == ST - 1),
                        )
                if is_last:
                    for h in range(2):
                        tt = g * 2 + h
                        nc.tensor.matmul(
                            otp_self,
                            vas[b][:, tt * 128:(tt + 1) * 128],
                            et[:, tt * 512:(tt + 1) * 512],
                            start=(tt == 0),
                            stop=(tt == ST - 1),
                        )
                    # incremental rowsum to shorten the drain
                    if g == 3:
                        nc.vector.tensor_add(
                            acch_last[:, :2048], et[:, :2048], et[:, 2048:4096]
                        )
                    elif g == 5:
                        nc.vector.tensor_add(
                            acch_last[:, :2048], acch_last[:, :2048],
                            et[:, 4096:6144],
                        )
            if idx == 0:
                for gp in range(1, 4):
                    emit_xt(0, 0, gp * 4, 4)
                    emit_proj(qTs[0], wq_sb, xTs[(0, 0)], 0, gp * 512, 512)
                emit_prep_v(0)
            # rowsum: DVE tree, all 16-bit to keep the 2x perf mode
            rowacc = rowbp.tile([128, 512], BF16, tag="rowb", name=f"row_{b}_{sc}")
            if is_last:
                nc.vector.tensor_add(
                    acch_last[:, :2048], acch_last[:, :2048], et[:, 6144:8192]
                )
                nc.vector.tensor_add(
                    acch_last[:, :1024], acch_last[:, :1024],
                    acch_last[:, 1024:2048],
                )
                nc.vector.tensor_add(
                    rowacc, acch_last[:, :512], acch_last[:, 512:1024]
                )
            else:
                # L1 on DVE (2x mode, 16-bit); L2-L4 on Pool (SBUF-only engine,
                # ~0.5 elem/cycle/lane but otherwise idle in steady state)
                acch = accp.tile([128, 4096], FP16, tag="acc", name=f"acch_{b}_{sc}")
                nc.vector.tensor_add(acch, et[:, :4096], et[:, 4096:])
                nc.gpsimd.tensor_add(acch[:, :2048], acch[:, :2048], acch[:, 2048:])
                nc.gpsimd.tensor_add(acch[:, :1024], acch[:, :1024],
                                     acch[:, 1024:2048])
                nc.gpsimd.tensor_add(rowacc, acch[:, :512], acch[:, 512:1024])
            if prev is not None:
                pending_tail = (prev[0], prev[1], prev[3], potp)
            prev = (b, sc, et, rowacc, vas[b])
        # drain: tails of the final two chunks; split their out DMAs across
        # both queues so the last transfers land in parallel
        pb, psc, pet, prow, pvall = prev
        if pending_tail is not None:
            osb6 = emit_tail_evict(pending_tail)
            emit_tail_rest(pending_tail, osb6, dma_engs=(nc.sync, nc.gpsimd))
        last_t = (pb, psc, prow, otp_self)
        osb7 = emit_tail_evict(last_t)
        emit_tail_rest(last_t, osb7, dma_engs=(nc.gpsimd, nc.sync))


_NC_CACHE = None


def _build():
    global _NC_CACHE
    if _NC_CACHE is not None:
        return _NC_CACHE
    nc = bacc.Bacc("TRN2", target_bir_lowering=False, debug=False, num_devices=NCORES)
    x1 = nc.dram_tensor("x_1", (S, BPC, DH), F32, kind="ExternalInput").ap()
    x2 = nc.dram_tensor("x_2", (S, BPC, DH), F32, kind="ExternalInput").ap()
    wq = nc.dram_tensor("Wq", (DH, DK), F32, kind="ExternalInput").ap()
    wk = nc.dram_tensor("Wk", (DH, DK), F32, kind="ExternalInput").ap()
    wv = nc.dram_tensor("Wv", (DH, DK), F32, kind="ExternalInput").ap()
    out = nc.dram_tensor("out", (S, BPC, DK), F32, kind="ExternalOutput").ap()
    with tile.TileContext(nc) as tc:
        _attention_kernel(tc, out, x1, x2, wq, wk, wv)
    nc.compile()
    _NC_CACHE = nc
    return nc


def _in_maps(x_1, x_2, Wq, Wk, Wv):
    maps = []
    for c in range(NCORES):
        bsl = slice(c * BPC, (c + 1) * BPC)
        maps.append({
            "x_1": np.ascontiguousarray(x_1[:, bsl, :], dtype=np.float32),
            "x_2": np.ascontiguousarray(x_2[:, bsl, :], dtype=np.float32),
            "Wq": np.asarray(Wq, dtype=np.float32),
            "Wk": np.asarray(Wk, dtype=np.float32),
            "Wv": np.asarray(Wv, dtype=np.float32),
        })
    return maps


def run(x_1, x_2, Wq, Wk, Wv, **spmd_kwargs):
    nc = _build()
    in_maps = _in_maps(x_1, x_2, Wq, Wk, Wv)
    last_err = None
    for _attempt in range(3):
        try:
            res = run_bass_kernel_spmd(
                nc, in_maps, core_ids=list(range(NCORES)), **spmd_kwargs
            )
            break
        except Exception as e:  # transient NRT_EXEC_UNIT_UNRECOVERABLE faults
            last_err = e
    else:
        raise last_err
    out = np.concatenate([res.results[c]["out"] for c in range(NCORES)], axis=1)
    return out, res


def kernel(x_1, x_2, Wq, Wk, Wv):
    out, _ = run(x_1, x_2, Wq, Wk, Wv)
    return out.astype(np.float32)
